# revision 33
# baseline (speedup 1.0000x reference)
"""Trainium2 Bass kernel: MoE top-k router (top-8 of 64 experts + softmax).

Contract: kernel(logits, top_k) takes the FULL inputs (logits [1048576, 64]
f32, top_k == 8) and returns (topk_idx int64 [N, 8], topk_w f32 [N, 8]),
matching jax.lax.top_k + jax.nn.softmax semantics (stable descending order,
ties broken toward the smaller index).

Sharding: data-parallel over tokens across 8 NeuronCores (one SPMD program,
per-core slices fed via run_bass_kernel_spmd). Per core, tokens are laid out
partition-major — partition p owns tokens [p*1024, (p+1)*1024).

Index-in-mantissa design (vs the earlier cascade+match op at ~137 DVE
cycles/token): each logit is first rewritten to a KEY that carries its own
expert index: key = (bits(x) & ~63) | j. Zeroing 6 low mantissa bits
perturbs the value by <= 2^-17 relative (harmless for the softmax) and
makes within-row keys pairwise distinct, so a plain 8-deep swap-flop MIN
cascade (64 DVE cycles/token, the 8-ALU lower bound; drains ride the next
token's ramp) yields the sorted top-8 keys, whose low 6 bits ARE the
indices. No match pass, no index drain.

The key transform is one fused scalar_tensor_tensor pass on the DVE
((x & -64) | code, int32 views) — bitwise ops are DVE-only on TRN2, and
the DVE write lane retires one 32-bit result/cycle, so transform+cascade
= 128 DVE cycles/token is this architecture's floor. exp on ScalarE; the
per-token sum as a pairwise add tree on GPSIMD; reciprocal on the DVE;
the broadcast scale + w store on GPSIMD (softmax tail emitted two tiles
deferred so the cross-queue chain never stalls the DVE ahead of the next
cascade). Outputs: sorted keys (f32; host extracts idx = bits & 63) and
softmax weights in bf16. Input DMAs own the Sync queue; y stores issue
from the Scalar queue (HWDGE) so descgen stays off the Q7s.

Masked ties: two row elements agreeing in their top 26 bits sort by code,
not by true value. Any wrong ordering/membership REQUIRES such a tie
touching the returned top-8, so the host flags rows where (a) two adjacent
returned keys share masked bits or (b) >=2 row elements share the 8th key's
masked bits, and recomputes those rows (~1e-3 of rows) exactly from the
logits it already holds. All other rows are bit-exact against jax order.
"""

import sys

if "/opt/trn_rl_repo" not in sys.path:
    sys.path.insert(0, "/opt/trn_rl_repo")

from dataclasses import dataclass

import numpy as np

N_TOKENS = 1048576
E = 64             # experts
K = 8              # top-k
NCORES = 8
P = 128            # SBUF partitions
TPC = N_TOKENS // NCORES   # tokens per core = 131072
TPP = TPC // P             # tokens per partition = 1024
T = 96                     # tokens per partition per full tile
# The key transform is DVE-only: TRN2 supports 32-bit bitwise ops on no
# other engine (NCC_EBIR039), and the DVE write lane retires one 32-bit
# result per cycle, so the fused scalar_tensor_tensor pass (1 elem/cycle)
# is the floor. (A dual-port 2x custom op was tried; the write lane can't
# retire 2x32b through the raw-uop path.)

_CACHE = {}


# --------------------------------------------------------------------------
# Custom cascade-only DVE op: sorted top-8 keys, 64 cycles/token.
#
# uOp chain (tok_len = 64):
#   0: RAMP0 entry (emits prev s0)               LT_8->epi, CNT1->2
#   1: RAMP0 loop  (same config; loop target)    LT_8->epi, CNT1->2
#   2..8: RAMP1..7 (MIN cascade j<k + swap-exchange at k, emits prev vk)
#   9: STEADY      (8-stage MIN cascade)         CNT(tok_len-8)->1 (next token)
#  10..16: epilogue VDRAIN s1..s7 for the final token -> IDLE
# --------------------------------------------------------------------------

def _build_cascade_uops(tok_len=64):
    from concourse.dve_uop import (
        AluInp, AluOp, InpSel, OutPath, OutSel, Trigger, UopConfig, ENABLE,
    )

    def ramp(k):
        """Element k of a token: MIN-cascade through stages < k, then a
        swap-EXCHANGE at stage k: BYPASS(a=CURR_SWAP_OUT, b=chain elem) emits
        the PREVIOUS token's (k+1)-th largest (alu_out = a) while latching the
        new seed (swap <- b), so the 8 key drains ride the next token's ramp
        for free. Stages > k forward the emitted value to the write port."""
        u = UopConfig()
        u.enable_input(InpSel.SRC_0, 0)
        u.require_inp0 = ENABLE
        u.repeat_count = 1
        for j in range(k):
            u.datapath_config[j].enable_alu(
                AluOp.MIN, AluInp.CURR_SWAP_OUT, AluInp.PREV_ALU_OUT
            )
            u.datapath_config[j].swap_enable = ENABLE
        u.datapath_config[k].enable_alu(
            AluOp.BYPASS, AluInp.CURR_SWAP_OUT, AluInp.PREV_ALU_OUT
        )
        u.datapath_config[k].swap_enable = ENABLE
        for j in range(k + 1, 8):
            u.datapath_config[j].pass_through_alu()
        u.enable_output(OutSel.ALU_OUT, OutPath.WR0_LO)
        return u

    uops = []
    # Termination: src0's AP carries ONE extra element past the last token, so
    # ramp0 always has data to issue (it never stalls on requires_src0) and
    # the level-evaluated SRC_TENSOR_LT_8 fires on that issue cycle -> epi.
    # (Waiting stalled on SRC_TENSOR_DONE after the stream drained misses the
    # done event and leaves the uOp FSM stalled past instruction retirement,
    # wedging the engine for the next NEFF execution.)
    for _ in (0, 1):  # 0: entry, 1: loop re-entry
        u = ramp(0)
        u.trigger = (Trigger.SRC_TENSOR_LT_8, Trigger.COUNT, Trigger.NONE)
        u.next_uop = (10, 2, 0)
        uops.append(u)
    for k in range(1, 8):
        u = ramp(k)
        u.trigger = (Trigger.COUNT, Trigger.NONE, Trigger.NONE)
        u.next_uop = (k + 2, 0, 0)
        uops.append(u)

    u = UopConfig()  # 9: steady (8-stage MIN cascade), then next token's ramp
    u.enable_input(InpSel.SRC_0, 0)
    u.require_inp0 = ENABLE
    u.repeat_count = tok_len - 8
    for j in range(8):
        u.datapath_config[j].enable_alu(
            AluOp.MIN, AluInp.PREV_ALU_OUT, AluInp.CURR_SWAP_OUT
        )
        u.datapath_config[j].swap_enable = ENABLE
    u.trigger = (Trigger.COUNT, Trigger.NONE, Trigger.NONE)
    u.next_uop = (1, 0, 0)
    uops.append(u)

    for k in range(1, 8):  # 10..16: epilogue value drains s1..s7 (last token;
        # its s0 was emitted by the ramp0 issue that took the LT_8 exit)
        u = UopConfig()
        u.repeat_count = 1
        u.datapath_config[k].enable_alu(
            AluOp.BYPASS, AluInp.CURR_SWAP_OUT, AluInp.CURR_SWAP_OUT
        )
        for j in range(k + 1, 8):
            u.datapath_config[j].pass_through_alu()
        u.enable_output(OutSel.ALU_OUT, OutPath.WR0_LO)
        u.trigger = (Trigger.COUNT, Trigger.NONE, Trigger.NONE)
        u.next_uop = (10 + k if k < 7 else 0, 0, 0)
        uops.append(u)
    return uops


# --------------------------------------------------------------------------
# Custom dual-port key-transform DVE op: out pair = ((x & ~63) | code) at
# 2 elements/cycle. Port0 = even elements, port1 = odd elements. Codes are
# generated internally: slice 0's ALU-out flop is a free-running pair
# counter k (seeded to -1 by uop 0), code_even = (k & 31) << 1,
# code_odd = code_even | 1 — tokens are 64 elements, so 32 pairs per token
# and the (k & 31) wrap lands exactly on token boundaries.
#
# Delay chains: D0=x_even(->code_odd) D1=x_odd(->masked_even) D2=mask(->
# masked_odd) D3=one(->out_even) D4=const31 D5=code_even.
# --------------------------------------------------------------------------

def _build_keyxf_uops():
    from concourse.dve_uop import (
        AluInp, AluOp, InpSel, OutPath, OutSel, Trigger, UopConfig, ENABLE,
        DelayInp,
    )

    def pair_uop():
        u = UopConfig()
        u.enable_input(InpSel.SRC_0, 1)    # x_even -> D0
        u.enable_input(InpSel.SRC_1, 2)    # x_odd  -> D1
        u.enable_input(InpSel.CONST_0, 3)  # mask 0xFFFFFFC0 (s0) -> D2
        u.enable_input(InpSel.ONE_U32, 4)  # 1 -> D3
        u.enable_input(InpSel.CONST_1, 5)  # 31 (s1) -> D4
        u.require_inp0 = ENABLE
        u.require_inp1 = ENABLE
        dp = u.datapath_config
        dp[0].enable_alu(AluOp.ADD, AluInp.CURR_ALU_OUT, AluInp.PREV_DELAY_3)
        dp[0].pass_through_delay(0, 1, 2, 3, 4)
        dp[1].enable_alu(
            AluOp.BITWISE_AND, AluInp.PREV_ALU_OUT, AluInp.PREV_DELAY_4
        )
        dp[1].pass_through_delay(0, 1, 2, 3)
        dp[2].enable_alu(
            AluOp.LOGICAL_SHIFT_LEFT, AluInp.PREV_ALU_OUT, AluInp.PREV_DELAY_3
        )
        dp[2].pass_through_delay(0, 1, 2, 3)
        dp[3].enable_alu(
            AluOp.BITWISE_OR, AluInp.PREV_ALU_OUT, AluInp.PREV_DELAY_3
        )
        dp[3].enable_delay_from_src(DelayInp.PREV_ALU_OUT, 5)  # D5 <- code_e
        dp[3].pass_through_delay(0, 1, 2)
        dp[4].enable_alu(
            AluOp.BITWISE_AND, AluInp.PREV_DELAY_0, AluInp.PREV_DELAY_2
        )
        dp[4].enable_delay_from_src(DelayInp.PREV_ALU_OUT, 0)  # D0 <- code_o
        dp[4].pass_through_delay(1, 2, 5)
        dp[5].enable_alu(
            AluOp.BITWISE_AND, AluInp.PREV_DELAY_1, AluInp.PREV_DELAY_2
        )
        dp[5].enable_delay_from_src(DelayInp.PREV_ALU_OUT, 1)  # D1 <- masked_e
        dp[5].pass_through_delay(0, 5)
        dp[6].enable_alu(
            AluOp.BITWISE_OR, AluInp.PREV_DELAY_1, AluInp.PREV_DELAY_5
        )
        dp[6].enable_delay_from_src(DelayInp.PREV_ALU_OUT, 2)  # D2 <- masked_o
        dp[6].pass_through_delay(0)
        dp[7].enable_alu(
            AluOp.BITWISE_OR, AluInp.PREV_DELAY_2, AluInp.PREV_DELAY_0
        )
        dp[7].enable_delay_from_src(DelayInp.PREV_ALU_OUT, 3)  # D3 <- out_e
        u.enable_output(OutSel.DELAY_3, OutPath.WR0_LO)        # out_even
        u.enable_output(OutSel.ALU_OUT, OutPath.WR0_HI)        # out_odd
        # write port 0 retires TWO results per cycle (LO+HI) — without this
        # the dst walker advances one element per cycle and the instruction
        # never completes
        u.force_two_data_zero = ENABLE
        return u

    u0 = UopConfig()  # 0: seed the pair counter (slice 0 ALU flop) to -1
    u0.enable_input(InpSel.ZERO, 1)
    u0.enable_input(InpSel.ONE_U32, 2)
    u0.repeat_count = 1
    u0.datapath_config[0].enable_alu(
        AluOp.SUBTRACT, AluInp.PREV_DELAY_0, AluInp.PREV_DELAY_1
    )
    u0.trigger = (Trigger.COUNT, Trigger.NONE, Trigger.NONE)
    u0.next_uop = (1, 0, 0)

    # Termination mirrors the cascade op: each port's AP carries ONE extra
    # element past the last token (one pad pair), so uop 1 always has data
    # to issue and the level-evaluated SRC_TENSOR_LT_8 fires on the pad
    # issue -> IDLE. (SRC_TENSOR_DONE is an edge event that races the port
    # prefetch and wedges the uop FSM past instruction retirement.)
    u1 = pair_uop()  # 1: first pair of a token (LT_8 checkpoint)
    u1.repeat_count = 1
    u1.trigger = (Trigger.SRC_TENSOR_LT_8, Trigger.COUNT, Trigger.NONE)
    u1.next_uop = (0, 2, 0)

    u2 = pair_uop()  # 2: pairs 1..31 of a token
    u2.repeat_count = 31
    u2.trigger = (Trigger.COUNT, Trigger.NONE, Trigger.NONE)
    u2.next_uop = (1, 0, 0)
    return [u0, u1, u2]


def _register_op(name_prefix, uops, rd1_en, spec):
    from concourse.dve_ops import (
        DveOp, OPS, CUSTOM_DVE_SPECS, _SUB_OPCODE_FOR_NAME, get_dve_sub_opcode,
    )
    from concourse.dve_uop import DveOpSpec

    # op name carries the uop-bytes hash: a uop edit changes the BIR and so
    # the NEFF cache key, preventing stale-table reuse.
    tag = DveOpSpec(name="probe", opcode=1, uops=uops, rd1_en=rd1_en).sha("v3")[:8]
    name = f"{name_prefix}_{tag}"

    @dataclass(frozen=True)
    class RawDveOp(DveOp):
        raw_uops: tuple = ()
        rd1: bool = False

        def compile(self, ver):
            assert ver == "v3", f"hand-written for TRN2/v3 only, got {ver}"
            return DveOpSpec(
                name=self.name,
                opcode=get_dve_sub_opcode(self.name),
                uops=list(self.raw_uops),
                rd1_en=self.rd1,
            )

    op = RawDveOp(
        name=name,
        spec=spec,
        subdim=False,
        uops_sha={},
        raw_uops=tuple(uops),
        rd1=rd1_en,
    )
    if name not in _SUB_OPCODE_FOR_NAME:
        row = max(_SUB_OPCODE_FOR_NAME.values()) + 1
        assert row < 0x20, f"row {row} overflows the 5-bit byte-36 field"
        OPS.append(op)
        CUSTOM_DVE_SPECS[op.name] = op.spec
        _SUB_OPCODE_FOR_NAME[op.name] = row
    return op


def _get_cascade_op():
    if "op" in _CACHE:
        return _CACHE["op"]
    from concourse.dve_spec import Spec, Src0

    def _ref(in0, in1, s0, s1, imm2):
        # CoreSim-only; the HW path never calls this. First 8 outputs are
        # stale swap flops on HW; zeros here.
        p = in0.shape[0]
        flat = in0.reshape(p, -1)
        t = (flat.shape[1] - 1) // E
        x = flat[:, : t * E].reshape(p, t, E)
        out = np.zeros((p, t * K + 8), dtype=np.float32)
        out[:, 8:] = (-np.sort(-x, axis=-1, kind="stable")[..., :K]).reshape(p, -1)
        return out

    op = _register_op(
        "TOP8K", _build_cascade_uops(E), False,
        Spec(body=Src0 + Src0, reference=_ref),
    )
    _CACHE["op"] = op
    return op


def _get_keyxf_op():
    if "xf" in _CACHE:
        return _CACHE["xf"]
    from concourse.dve_spec import Spec, Src0, Src1

    def _ref(in0, in1, s0, s1, imm2):
        # CoreSim-only. in0 = even elements, in1 = odd; output interleaved.
        p, h = in0.shape[0], in0.reshape(in0.shape[0], -1).shape[1]
        ev = in0.reshape(p, h).view(np.uint32)
        od = in1.reshape(p, h).view(np.uint32)
        ce = (2 * np.arange(h, dtype=np.uint32)) & np.uint32(63)
        out = np.zeros((p, 2 * h), dtype=np.uint32)
        out[:, 0::2] = (ev & np.uint32(0xFFFFFFC0)) | ce[None, :]
        out[:, 1::2] = (od & np.uint32(0xFFFFFFC0)) | (ce + 1)[None, :]
        return out.view(np.float32)

    op = _register_op(
        "KEYXF", _build_keyxf_uops(), True,
        Spec(body=Src0 + Src1, reference=_ref),
    )
    _CACHE["xf"] = op
    return op


def _build(tpp=TPP, t_tile=T):
    import concourse.bacc as bacc
    import concourse.mybir as mybir
    import concourse.tile as tile

    f32 = mybir.dt.float32
    i32 = mybir.dt.int32
    bf16 = mybir.dt.bfloat16
    op = _get_cascade_op()

    n_tok = P * tpp
    # small first tiles (the cascade starts after ~0.5MB of DMA instead of
    # 3MB) and small last tiles (short softmax/store tail after the final
    # cascade instruction)
    if tpp == 1024:
        sizes = [8, 24, 64] + [96] * 9 + [32, 16, 16]
    else:
        sizes = [t_tile] * (tpp // t_tile)
    assert sum(sizes) == tpp
    offs = [sum(sizes[:j]) for j in range(len(sizes))]

    nc = bacc.Bacc("TRN2", target_bir_lowering=False, debug=False)
    logits = nc.dram_tensor("logits", [n_tok, E], f32, kind="ExternalInput")
    # y_out row = sorted top-8 keys; low 6 bits of each = expert index.
    y_out = nc.dram_tensor("y_out", [n_tok, K], f32, kind="ExternalOutput")
    w_out = nc.dram_tensor("w_out", [n_tok, K], bf16, kind="ExternalOutput")

    lg_v = logits.ap().rearrange("(p t) e -> p t e", p=P, t=tpp)
    y_v = y_out.ap().rearrange("(p t) k -> p t k", p=P, t=tpp)
    w_v = w_out.ap().rearrange("(p t) k -> p t k", p=P, t=tpp)

    with tile.TileContext(nc) as tc:
        with tc.tile_pool(name="io", bufs=5) as pool:
            # constants: per-position index code 0..63 and the mantissa mask
            code_t = pool.tile([P, 1, E], i32, tag="code")
            nc.gpsimd.iota(code_t[:], pattern=[[1, E]], base=0,
                           channel_multiplier=0)
            mask_t = pool.tile([P, 1], i32, tag="mask")
            nc.gpsimd.memset(mask_t[:], -64)  # 0xFFFFFFC0

            def softmax_tail(tt, o, y, ex, drain=False):
                """Softmax tail, emitted two tiles deferred so its inputs are
                always ready and these ops never stall any queue ahead of the
                next tiles' transform/cascade. reduce + recip ride the DVE
                queue; the broadcast scale and the w store (SWDGE descgen)
                ride the Pool queue back-to-back."""
                # per-token sum as a pairwise add tree on the Pool queue
                # (gpsimd tensor_reduce can't reduce free axes; this keeps
                # the ~0.8us/tile segmented reduce off the saturated DVE)
                a = pool.tile([P, tt, 4], f32, tag="sa")
                nc.gpsimd.tensor_add(a[:], ex[:, :, 0:4], ex[:, :, 4:8])
                b = pool.tile([P, tt, 2], f32, tag="sb")
                nc.gpsimd.tensor_add(b[:], a[:, :, 0:2], a[:, :, 2:4])
                s = pool.tile([P, tt, 1], f32, tag="s")
                nc.gpsimd.tensor_add(s[:], b[:, :, 0:1], b[:, :, 1:2])
                r = pool.tile([P, tt, 1], f32, tag="r")
                # ~51-ULP single-pass approx (exact divide iterates 8 cycles
                # per element); 4e-6 relative on w, far inside the tolerance.
                # (A single Pool divide would fuse this with the scale, but
                # walrus rejects divide on Pool: NCC_IXCG966.)
                nc.vector.reciprocal_approx_fast(r[:], s[:])
                w = pool.tile([P, tt, K], bf16, tag="w")
                nc.gpsimd.tensor_mul(w[:], ex[:], r[:].broadcast_to([P, tt, K]))
                if drain:
                    # post-loop tails: the Sync queue is idle (all input
                    # loads issued), and HWDGE setup beats SWDGE descgen
                    nc.sync.dma_start(w_v[:, o:o + tt, :], w[:])
                else:
                    nc.gpsimd.dma_start(w_v[:, o:o + tt, :], w[:])

            pending = []
            for o, tt in zip(offs, sizes):
                # one extra trailing element feeds the LT_8 termination issue
                n = tt * E
                x = pool.tile([P, n + 1], f32, tag="x")
                x3 = x[:, 0:n].rearrange("p (t e) -> p t e", t=tt, e=E)
                nc.sync.dma_start(x3, lg_v[:, o:o + tt, :])
                # key transform, in place: x <- (x & ~63) | code. One fused
                # DVE pass; the write port retires one 32-bit result per
                # cycle, so no elementwise DVE op can beat 1 elem/cycle.
                xi = x[:, 0:n].bitcast(i32).rearrange(
                    "p (t e) -> p t e", t=tt, e=E
                )
                nc.vector.scalar_tensor_tensor(
                    xi, xi, mask_t[:],
                    code_t[:].broadcast_to([P, tt, E]),
                    op0=mybir.AluOpType.bitwise_and,
                    op1=mybir.AluOpType.bitwise_or,
                )
                # output stream: 8 garbage words (stale swap flops emitted
                # by the first token's ramp), then per token 8 sorted keys
                yr = pool.tile([P, tt * K + 8], f32, tag="y")
                y = yr[:, 8:].rearrange("p (t k) -> p t k", t=tt, k=K)
                nc.vector._custom_dve(op, out=yr[:], in0=x[:], s0=0.0, s1=0.0)
                ex = pool.tile([P, tt, K], f32, tag="ex")
                nc.scalar.activation(
                    ex[:], y, mybir.ActivationFunctionType.Exp
                )
                # y store from the Scalar queue (HWDGE): input loads own the
                # Sync queue; descriptor generation stays off the Q7s.
                nc.scalar.dma_start(y_v[:, o:o + tt, :], y)
                pending.append((tt, o, y, ex))
                if len(pending) >= 3:
                    softmax_tail(*pending.pop(0))
            for args in pending:
                softmax_tail(*args, drain=True)
    nc.compile()
    return nc


def _get_nc():
    if "nc" not in _CACHE:
        _CACHE["nc"] = _build()
    return _CACHE["nc"]


def _bf16_to_f32(a):
    """bf16 -> f32 without depending on ml_dtypes: widen the bit pattern."""
    u = np.asarray(a)
    if u.dtype == np.float32:
        return u
    u16 = u.view(np.uint16)
    return (u16.astype(np.uint32) << 16).view(np.float32)


def kernel(logits, top_k):
    logits = np.ascontiguousarray(np.asarray(logits, dtype=np.float32))
    k = int(np.asarray(top_k))
    assert k == K, f"kernel hardcodes top_k={K}, got {k}"
    assert logits.shape == (N_TOKENS, E), logits.shape

    from concourse.bass_utils import run_bass_kernel_spmd

    nc = _get_nc()
    chunks = logits.reshape(NCORES, TPC, E)
    in_maps = [{"logits": np.ascontiguousarray(chunks[c])} for c in range(NCORES)]
    # The tunneled devices occasionally fail a run with a transient
    # NRT_EXEC_UNIT_UNRECOVERABLE error; a wedged device self-heals when the
    # relay resets it on reclaim, which takes minutes — back off accordingly.
    last_err = None
    for _attempt, delay_s in enumerate([5.0, 30.0, 60.0, 90.0, 120.0]):
        try:
            res = run_bass_kernel_spmd(nc, in_maps, list(range(NCORES)))
            break
        except Exception as e:  # noqa: BLE001 - retry transient device faults
            last_err = e
            import time as _time

            _time.sleep(delay_s)
    else:
        raise last_err

    # Row r of each per-core output is token r of that core's slice, so a
    # plain concat along the token axis reassembles the full outputs.
    y = np.concatenate([r["y_out"] for r in res.results], axis=0)
    w = _bf16_to_f32(
        np.concatenate([r["w_out"] for r in res.results], axis=0)
    ).astype(np.float32)

    kb = y.view(np.uint32)
    idx = (kb & np.uint32(63)).astype(np.int64)

    # --- masked-tie repair -------------------------------------------------
    # A wrong ordering or membership requires two row elements whose top-26
    # bits collide AND that touch the returned top-8. Flag rows where (a)
    # two adjacent returned keys share masked bits, or (b) >=2 row elements
    # share the 8th returned key's masked bits; recompute those exactly.
    m = np.uint32(0xFFFFFFC0)
    mk = kb & m
    flag = (mk[:, :-1] == mk[:, 1:]).any(axis=1)
    xb = logits.view(np.uint32) & m
    flag |= (xb == mk[:, K - 1:K]).sum(axis=1) >= 2
    rows = np.nonzero(flag)[0]
    if rows.size:
        v = logits[rows]                              # [m, 64]
        order = np.argsort(-v, axis=-1, kind="stable")[:, :K]
        idx[rows] = order
        tv = np.take_along_axis(v, order, axis=-1).astype(np.float64)
        ex = np.exp(tv - tv.max(axis=-1, keepdims=True))
        w[rows] = (ex / ex.sum(axis=-1, keepdims=True)).astype(np.float32)
    return idx, w
